# revision 61
# baseline (speedup 1.0000x reference)
"""Trainium2 Bass kernel for CausalSelectiveSelfAttentionWithMemoryPenalty.

Reference computation (B=2, T=2048, C=768, nh=12, hd=64, tau=1.0):
    qkv = x @ w_attn + b_attn ; q,k,v per head
    att = causal_masked(q k^T / 8)
    S = relu(att[:,0]) with col 0 and diagonal zeroed
    FF[q,t] = sum_{r<q} S[r,t]          (cumsum of row-shifted S)
    FF_sum[q] = sum_t clip(FF[q,t],0,1) ;  M[0,b,i,j] = i - FF_sum[b,j]
    y = softmax(att - FF) @ v ; y = y @ w_proj + b_proj
    returns (y, M)

Sharding: 8 cores = 2 batches x 4 head-groups (3 heads each). Each core
recomputes head-0 scores for its batch (aux slot) to build FF locally —
no collectives. Attention is computed transposed (attT[t,q]) so that
softmax normalization comes from an appended ones-column of v and the
AV matmul consumes P_T directly as the moving operand. M is built on
host from FF_sum (tiny) instead of shipping 33MB from the device.

Numerics: matmul/elementwise P path in bf16 (PE 1 cyc/row; DVE
tensor_tensor 2x mode is bf16-only), FF/scan chain in fp32. qkv and v
tensors are split into per-512-column tiles so the Tile scheduler can
overlap qkv production with early attention chunks (keeps the PE dense
enough that the HAM clock gate stays open).
"""
import sys

if "/opt/trn_rl_repo" not in sys.path:
    sys.path.insert(0, "/opt/trn_rl_repo")

import numpy as np

import concourse.bass as bass
from concourse import bacc, mybir
from concourse.tile import TileContext
from concourse.bass_utils import run_bass_kernel_spmd

F32 = mybir.dt.float32
BF16 = mybir.dt.bfloat16

B, T, C = 2, 2048, 768
NH, HD = 12, 64
TAU = 1.0
HPG = 3           # heads per group
NKC = C // 128    # 6 contraction chunks for qkv
NQG = T // 512    # 4 q-groups of 512
NTC = T // 128    # 16 t-chunks of 128


def _build_nc():
    nc = bacc.Bacc()

    xT = nc.dram_tensor("xT", [128, NKC, T], BF16, kind="ExternalInput")
    wl = nc.dram_tensor("wl", [128, NKC, 768], BF16, kind="ExternalInput")
    bcol = nc.dram_tensor("bcol", [128, NKC], F32, kind="ExternalInput")
    wp01 = nc.dram_tensor("wp01", [128, 768], BF16, kind="ExternalInput")
    wp2 = nc.dram_tensor("wp2", [64, 768], BF16, kind="ExternalInput")
    m_strict = nc.dram_tensor("m_strict", [128, 128], F32, kind="ExternalInput")
    m_bias = nc.dram_tensor("m_bias", [128, 128], BF16, kind="ExternalInput")
    ones_in = nc.dram_tensor("ones_in", [128, HPG * NTC], BF16, kind="ExternalInput")
    vbias = nc.dram_tensor("vbias", [128, 192], F32, kind="ExternalInput")

    YP = nc.dram_tensor("ypart", [T, 768], BF16, kind="ExternalOutput")
    FFS = nc.dram_tensor("ffsum", [1, T], F32, kind="ExternalOutput")

    with TileContext(nc) as tc:
        dma = nc.sync
        with tc.tile_pool(name="const", bufs=1) as constp, \
             tc.tile_pool(name="pers", bufs=1) as pers, \
             tc.tile_pool(name="ph2sb", bufs=4) as p2, \
             tc.tile_pool(name="ptsb", bufs=8) as p2b, \
             tc.tile_pool(name="zsb", bufs=2) as zsb, \
             tc.tile_pool(name="osb", bufs=2) as osb, \
             tc.tile_pool(name="aps", bufs=4, space="PSUM") as aps, \
             tc.tile_pool(name="yps", bufs=1, space="PSUM") as yps, \
             tc.tile_pool(name="fpp", bufs=1, space="PSUM") as fpp:
            ms_sb = constp.tile([128, 128], F32)
            mi_sb = constp.tile([128, 128], BF16)
            ones_sb = constp.tile([128, 1], BF16)
            wp01_sb = constp.tile([128, 768], BF16)
            wp2_sb = constp.tile([64, 768], BF16)
            bcol_sb = constp.tile([128, NKC], F32)
            vb_sb = constp.tile([128, 192], F32)

            # persistent state; qkv/vaug split per 512-wide t-chunk so early
            # attention chunks only depend on the slices already produced
            qkv_n = [pers.tile([128, 4, 512], BF16, name=f"qkv{n}")
                     for n in range(4)]
            vaug_n = [pers.tile([128, HPG, 4, 65], BF16, name=f"vaug{n}")
                      for n in range(4)]
            xT_sb = pers.tile([128, NKC, T], BF16)
            wl_sb = pers.tile([128, NKC, 768], BF16)
            yT01 = pers.tile([128, T], BF16)          # heads 0,1 (d stacked)
            yT2 = pers.tile([64, T], BF16)            # head 2
            accs = pers.tile([128, NTC], F32)         # cumsum carry per t-chunk
            ffs_sb = pers.tile([1, T], F32)
            nc.vector.memset(accs, 0.0)
            # input DMAs spread over both HWDGE queues in consumption order:
            # weights + first x chunk first, early-needed constants, then the
            # remaining x chunks and late-use constants
            qs = [nc.sync, nc.scalar]
            for kc in range(NKC):
                qs[kc % 2].dma_start(
                    out=wl_sb[:, kc, :], in_=wl[:, kc, :])
            for kc in range(NKC):
                qs[kc % 2].dma_start(
                    out=xT_sb[:, kc, 0:512], in_=xT[:, kc, 0:512])
            dma.dma_start(out=bcol_sb, in_=bcol[:, :])
            nc.scalar.dma_start(out=vb_sb, in_=vbias[:, :])
            dma.dma_start(out=ms_sb, in_=m_strict[:, :])
            nc.scalar.dma_start(out=mi_sb, in_=m_bias[:, :])
            dma.dma_start(out=ones_sb, in_=ones_in[:, 0:1])
            for n in range(4):
                nin = ones_in[:, 12 * n:12 * n + 12]
                nin = nin.rearrange("p (a b) -> p a b", a=HPG).unsqueeze(3)
                qs[n % 2].dma_start(out=vaug_n[n][:, :, :, 64:65], in_=nin)
            for n in range(1, 4):
                for kc in range(NKC):
                    qs[(n * NKC + kc) % 2].dma_start(
                        out=xT_sb[:, kc, n * 512:(n + 1) * 512],
                        in_=xT[:, kc, n * 512:(n + 1) * 512])
            dma.dma_start(out=wp01_sb, in_=wp01[:, :])
            nc.scalar.dma_start(out=wp2_sb, in_=wp2[:, :])

            def col(c0, n):
                m, p = divmod(c0, 128)
                return qkv_n[n][p:p + 64, m, :]

            # column layout pairs each matmul's lhsT/rhs at the same
            # partition base (hardware constraint)
            def qT(h, n):
                return col((0, 64, 256)[h], n)

            def kT(h, n):
                return col((128, 192, 384)[h], n)

            def q0T(n):
                return col(320, n)

            def k0T(n):
                return col(448, n)

            # ---- phase 1: q/k in [col, t] layout; v directly in natural
            # [t, d] layout (lhsT = xT chunk), all with bias folded in
            def emit_qkv_piece(n, piece):
                if piece < 4:
                    m = piece
                    qkv_ps = aps.tile([128, 512], F32, tag="att", name="qkv_ps")
                    for kc in range(NKC):
                        nc.tensor.matmul(
                            qkv_ps,
                            wl_sb[:, kc, m * 128:(m + 1) * 128],
                            xT_sb[:, kc, n * 512:(n + 1) * 512],
                            start=(kc == 0), stop=(kc == NKC - 1))
                    nc.scalar.activation(
                        qkv_n[n][:, m, :], qkv_ps,
                        mybir.ActivationFunctionType.Identity,
                        bias=bcol_sb[:, m:m + 1], scale=1.0)
                else:
                    ii = piece - 4
                    t0 = (4 * n + ii) * 128
                    v_ps = aps.tile([128, 192], F32, tag="att", name="v_ps")
                    for kc in range(NKC):
                        nc.tensor.matmul(
                            v_ps, xT_sb[:, kc, t0:t0 + 128],
                            wl_sb[:, kc, 512:704],
                            start=(kc == 0), stop=(kc == NKC - 1))
                    nc.vector.tensor_add(
                        vaug_n[n][:, :, ii, 0:64],
                        v_ps.rearrange("p (h d) -> p h d", h=HPG),
                        vb_sb.rearrange("p (h d) -> p h d", h=HPG))

            def emit_qkv(n):
                for piece in range(8):
                    emit_qkv_piece(n, piece)

            emit_qkv(0)

            # ---- phase 2: attention + FF, transposed layout ----
            # qkv production of chunk n+1 is emitted between attention
            # q-groups so the PE always has dense independent matmul work
            # (keeps the HAM clock gate open)
            for jg in range(NQG):
                nt = 4 * jg + 4
                y_ps = [yps.tile([65, 512], F32, tag=f"y{h}", name=f"y_ps{h}")
                        for h in range(HPG)]
                ff_ps = fpp.tile([128, 512], F32, tag="fp", name="ff_ps")
                prev = None   # (i, qsl, ni, ti, pts, clip) pending AV/ff matmuls

                def emit_delayed(pi, pqsl, pni, pti, ppts, pclip):
                    nc.tensor.matmul(ff_ps[0:1, pqsl], ones_sb, pclip[:, pqsl],
                                     start=(pi == 0), stop=(pi == nt - 1))
                    for h in range(HPG):
                        nc.tensor.matmul(y_ps[h][:, pqsl],
                                         vaug_n[pni][:, h, pti, :],
                                         ppts[h][:, pqsl],
                                         start=(pi == 0), stop=(pi == nt - 1))

                for i in range(nt):
                    # interleave next q-group's qkv production piecewise so
                    # the PE has dense filler work between dependent matmuls
                    if jg + 1 < 4:
                        nk = min(nt, 8)
                        for piece in range(min(8, 8 * i // nk),
                                           min(8, 8 * (i + 1) // nk)):
                            emit_qkv_piece(jg + 1, piece)
                    s0 = max(0, i - 4 * jg)
                    qoff = s0 * 128
                    qsl = slice(qoff, 512)        # local q cols within group
                    ni, ti = divmod(i, 4)
                    tsl = slice(ti * 128, (ti + 1) * 128)
                    # aux: head-0 scores -> S_T -> FF via scan
                    a0 = aps.tile([128, 512], F32, tag="att", name="a0")
                    nc.tensor.matmul(a0[:, qsl], k0T(ni)[:, tsl],
                                     q0T(jg)[:, qsl], start=True, stop=True)
                    st = p2.tile([128, 512], F32, tag="st")
                    nc.vector.tensor_scalar_max(st[:, qsl], a0[:, qsl], 0.0)
                    if i >= 4 * jg:
                        nc.vector.tensor_mul(st[:, qoff:qoff + 128],
                                             st[:, qoff:qoff + 128], ms_sb)
                    if i == 0:
                        nc.gpsimd.memset(st[0:1, :], 0.0)
                    incl = p2.tile([128, 513], F32, tag="incl")
                    nc.gpsimd.tensor_copy(incl[:, qoff:qoff + 1], accs[:, i:i + 1])
                    nc.vector.tensor_tensor_scan(
                        incl[:, qoff + 1:513], st[:, qsl], st[:, qsl],
                        initial=incl[:, qoff:qoff + 1],
                        op0=mybir.AluOpType.add, op1=mybir.AluOpType.bypass)
                    nc.gpsimd.tensor_copy(accs[:, i:i + 1], incl[:, 512:513])
                    enf = p2b.tile([128, 512], BF16, tag="enf")
                    nc.scalar.activation(enf[:, qsl], incl[:, qoff:512],
                                         mybir.ActivationFunctionType.Exp,
                                         scale=-1.0)
                    if i >= 4 * jg:
                        nc.vector.tensor_mul(enf[:, qoff:qoff + 128],
                                             enf[:, qoff:qoff + 128], mi_sb)
                    clip = p2b.tile([128, 512], BF16, tag="clip")
                    nc.vector.tensor_scalar_min(clip[:, qsl], incl[:, qoff:512], 1.0)
                    # heads: QK matmuls + exp + enf-mult; AV/ff of the
                    # previous chunk are emitted between them so the PE
                    # always has independent work during the ACT/DVE
                    # roundtrip (keeps the HAM clock gate open)
                    pts = []
                    for h in range(HPG):
                        ah = aps.tile([128, 512], F32, tag="att", name="ah")
                        nc.tensor.matmul(ah[:, qsl], kT(h, ni)[:, tsl],
                                         qT(h, jg)[:, qsl], start=True, stop=True)
                        pt = p2b.tile([128, 512], BF16, tag="pt")
                        nc.scalar.activation(pt[:, qsl], ah[:, qsl],
                                             mybir.ActivationFunctionType.Exp)
                        nc.vector.tensor_mul(pt[:, qsl], pt[:, qsl], enf[:, qsl])
                        pts.append(pt)
                    if prev is not None:
                        emit_delayed(*prev)
                    prev = (i, qsl, ni, ti, pts, clip)
                emit_delayed(*prev)
                prev = None
                # normalize + stash ffsum
                nc.vector.tensor_copy(ffs_sb[0:1, jg * 512:(jg + 1) * 512],
                                      ff_ps[0:1, :])
                for h in range(HPG):
                    zrow = zsb.tile([1, 512], F32, tag="zrow")
                    nc.scalar.copy(zrow, y_ps[h][64:65, :])
                    zi = zsb.tile([1, 512], F32, tag="zi")
                    nc.vector.reciprocal_approx_fast(zi, zrow)
                    zb = zsb.tile([64, 512], F32, tag="zb")
                    nc.gpsimd.partition_broadcast(zb, zi)
                    if h < 2:
                        dst = yT01[64 * h:64 * h + 64, jg * 512:(jg + 1) * 512]
                    else:
                        dst = yT2[:, jg * 512:(jg + 1) * 512]
                    nc.vector.tensor_mul(dst, y_ps[h][0:64, :], zb)
                # ---- proj for this q-group ----
                for s in range(4):
                    qt = jg * 4 + s
                    qsl2 = slice(qt * 128, (qt + 1) * 128)
                    out_sb = osb.tile([128, 768], BF16, tag="out")
                    for n0, nw in ((0, 512), (512, 256)):
                        pp = fpp.tile([128, 512], F32, tag="fp", name="pp")
                        nc.tensor.matmul(pp[:, 0:nw], yT01[:, qsl2],
                                         wp01_sb[:, n0:n0 + nw],
                                         start=True, stop=False)
                        nc.tensor.matmul(pp[:, 0:nw], yT2[:, qsl2],
                                         wp2_sb[:, n0:n0 + nw],
                                         start=False, stop=True)
                        nc.scalar.copy(out_sb[:, n0:n0 + nw], pp[:, 0:nw])
                    dma.dma_start(out=YP[qsl2, :], in_=out_sb)
            dma.dma_start(out=FFS[:, :], in_=ffs_sb)
    nc.finalize()
    return nc


_NC = None


def _get_nc():
    global _NC
    if _NC is None:
        _NC = _build_nc()
    return _NC


def _prep_inputs(x, w_attn, b_attn, w_proj):
    f32 = np.float32
    bf16 = np.dtype("bfloat16") if hasattr(np, "bfloat16") else None
    import ml_dtypes
    bf16 = ml_dtypes.bfloat16
    tril = np.tril(np.ones((128, 128), f32), k=-1)
    m_strict = np.ascontiguousarray(tril.T)          # 1 where col > row
    m_bias = np.ascontiguousarray(np.triu(np.ones((128, 128), f32))).astype(bf16)
    ones_in = np.ones((128, HPG * NTC), f32).astype(bf16)

    in_maps = []
    for c in range(8):
        b, g = divmod(c, 4)

        def qc(h):
            return slice(64 * (3 * g + h), 64 * (3 * g + h) + 64)

        def kc_(h):
            return slice(768 + 64 * (3 * g + h), 768 + 64 * (3 * g + h) + 64)

        def vc(h):
            return slice(1536 + 64 * (3 * g + h), 1536 + 64 * (3 * g + h) + 64)

        wl = np.zeros((768, 768), f32)
        bl = np.zeros((768,), f32)
        # column layout: [q0 q1 | k0 k1 | q2 aux_q0 | k2 aux_k0 | v0 v1 | v2 pad]
        pieces = [
            (0, qc(0), 0.125), (64, qc(1), 0.125),
            (128, kc_(0), 1.0), (192, kc_(1), 1.0),
            (256, qc(2), 0.125), (320, slice(0, 64), 0.125),
            (384, kc_(2), 1.0), (448, slice(768, 832), 1.0),
            (512, vc(0), 1.0), (576, vc(1), 1.0), (640, vc(2), 1.0),
        ]
        for dst, src, scale in pieces:
            wl[:, dst:dst + 64] = w_attn[:, src] * scale
            bl[dst:dst + 64] = b_attn[src] * scale
        in_maps.append({
            "xT": np.ascontiguousarray(
                x[b].T.reshape(NKC, 128, T).transpose(1, 0, 2)).astype(bf16),
            "wl": np.ascontiguousarray(
                wl.reshape(NKC, 128, 768).transpose(1, 0, 2)).astype(bf16),
            "bcol": np.ascontiguousarray(bl.reshape(NKC, 128).T),
            "wp01": w_proj[192 * g:192 * g + 128, :].astype(bf16),
            "wp2": w_proj[192 * g + 128:192 * g + 192, :].astype(bf16),
            "m_strict": m_strict,
            "m_bias": m_bias,
            "ones_in": ones_in,
            "vbias": np.ascontiguousarray(
                np.broadcast_to(bl[512:704], (128, 192))),
        })
    return in_maps


def run_device(x, w_attn, b_attn, w_proj, trace=False, trace_kwargs=None):
    nc = _get_nc()
    in_maps = _prep_inputs(x, w_attn, b_attn, w_proj)
    res = run_bass_kernel_spmd(nc, in_maps, core_ids=list(range(8)),
                               trace=trace, **(trace_kwargs or {}))
    return res


def kernel(x, w_attn, b_attn, w_proj, b_proj):
    x = np.asarray(x, np.float32)
    w_attn = np.asarray(w_attn, np.float32)
    b_attn = np.asarray(b_attn, np.float32)
    w_proj = np.asarray(w_proj, np.float32)
    b_proj = np.asarray(b_proj, np.float32)

    res = run_device(x, w_attn, b_attn, w_proj)
    y = np.zeros((B, T, 768), np.float32)
    ffsum = np.zeros((B, T), np.float32)
    for c in range(8):
        b = c // 4
        y[b] += res.results[c]["ypart"].astype(np.float32)
        if c % 4 == 0:
            ffsum[b] = res.results[c]["ffsum"][0]
    y += b_proj
    pos = np.arange(T, dtype=np.float32).reshape(1, 1, T, 1)
    M = pos - (ffsum / TAU).reshape(1, B, 1, T)
    return (y, M)


# revision 63
# speedup vs baseline: 1.1243x; 1.1243x over previous
"""Trainium2 Bass kernel for CausalSelectiveSelfAttentionWithMemoryPenalty.

Reference computation (B=2, T=2048, C=768, nh=12, hd=64, tau=1.0):
    qkv = x @ w_attn + b_attn ; q,k,v per head
    att = causal_masked(q k^T / 8)
    S = relu(att[:,0]) with col 0 and diagonal zeroed
    FF[q,t] = sum_{r<q} S[r,t]          (cumsum of row-shifted S)
    FF_sum[q] = sum_t clip(FF[q,t],0,1) ;  M[0,b,i,j] = i - FF_sum[b,j]
    y = softmax(att - FF) @ v ; y = y @ w_proj + b_proj
    returns (y, M)

Sharding: 8 cores = 2 batches x 4 head-groups (3 heads each). Each core
recomputes head-0 scores for its batch (aux slot) to build FF locally —
no collectives. Attention is computed transposed (attT[t,q]) so that
softmax normalization comes from an appended ones-column of v and the
AV matmul consumes P_T directly as the moving operand. M is built on
host from FF_sum (tiny) instead of shipping 33MB from the device.

Numerics: matmul/elementwise P path in bf16 (PE 1 cyc/row; DVE
tensor_tensor 2x mode is bf16-only), FF/scan chain in fp32. qkv and v
tensors are split into per-512-column tiles so the Tile scheduler can
overlap qkv production with early attention chunks (keeps the PE dense
enough that the HAM clock gate stays open).
"""
import sys

if "/opt/trn_rl_repo" not in sys.path:
    sys.path.insert(0, "/opt/trn_rl_repo")

import numpy as np

import concourse.bass as bass
from concourse import bacc, mybir
from concourse.tile import TileContext
from concourse.bass_utils import run_bass_kernel_spmd

F32 = mybir.dt.float32
BF16 = mybir.dt.bfloat16

B, T, C = 2, 2048, 768
NH, HD = 12, 64
TAU = 1.0
HPG = 3           # heads per group
NKC = C // 128    # 6 contraction chunks for qkv
NQG = T // 512    # 4 q-groups of 512
NTC = T // 128    # 16 t-chunks of 128


def _build_nc():
    nc = bacc.Bacc()

    xT = nc.dram_tensor("xT", [128, NKC, T], BF16, kind="ExternalInput")
    wl = nc.dram_tensor("wl", [128, NKC, 768], BF16, kind="ExternalInput")
    bcol = nc.dram_tensor("bcol", [128, NKC], F32, kind="ExternalInput")
    wp01 = nc.dram_tensor("wp01", [128, 768], BF16, kind="ExternalInput")
    wp2 = nc.dram_tensor("wp2", [64, 768], BF16, kind="ExternalInput")
    m_strict = nc.dram_tensor("m_strict", [128, 128], F32, kind="ExternalInput")
    m_bias = nc.dram_tensor("m_bias", [128, 128], BF16, kind="ExternalInput")
    ones_in = nc.dram_tensor("ones_in", [128, HPG * NTC], BF16, kind="ExternalInput")
    vbias = nc.dram_tensor("vbias", [128, 192], F32, kind="ExternalInput")

    YP = nc.dram_tensor("ypart", [T, 768], BF16, kind="ExternalOutput")
    FFS = nc.dram_tensor("ffsum", [1, T], F32, kind="ExternalOutput")

    with TileContext(nc) as tc:
        dma = nc.sync
        with tc.tile_pool(name="const", bufs=1) as constp, \
             tc.tile_pool(name="pers", bufs=1) as pers, \
             tc.tile_pool(name="ph2sb", bufs=4) as p2, \
             tc.tile_pool(name="ptsb", bufs=8) as p2b, \
             tc.tile_pool(name="zsb", bufs=2) as zsb, \
             tc.tile_pool(name="osb", bufs=2) as osb, \
             tc.tile_pool(name="aps", bufs=4, space="PSUM") as aps, \
             tc.tile_pool(name="yps", bufs=1, space="PSUM") as yps, \
             tc.tile_pool(name="fpp", bufs=1, space="PSUM") as fpp:
            ms_sb = constp.tile([128, 128], F32)
            mi_sb = constp.tile([128, 128], BF16)
            ones_sb = constp.tile([128, 1], BF16)
            wp01_sb = constp.tile([128, 768], BF16)
            wp2_sb = constp.tile([64, 768], BF16)
            bcol_sb = constp.tile([128, NKC], F32)
            vb_sb = constp.tile([128, 192], F32)

            # persistent state; qkv/vaug split per 512-wide t-chunk so early
            # attention chunks only depend on the slices already produced
            qkv_n = [pers.tile([128, 4, 512], BF16, name=f"qkv{n}")
                     for n in range(4)]
            vaug_n = [pers.tile([128, HPG, 4, 65], BF16, name=f"vaug{n}")
                      for n in range(4)]
            xT_sb = pers.tile([128, NKC, T], BF16)
            wl_sb = pers.tile([128, NKC, 768], BF16)
            yT01 = pers.tile([128, T], BF16)          # heads 0,1 (d stacked)
            yT2 = pers.tile([64, T], BF16)            # head 2
            accs = pers.tile([128, NTC], F32)         # cumsum carry per t-chunk
            ffs_sb = pers.tile([1, T], F32)
            nc.vector.memset(accs, 0.0)
            # input DMAs spread over both HWDGE queues in consumption order:
            # weights + first x chunk first, early-needed constants, then the
            # remaining x chunks and late-use constants
            qs = [nc.sync, nc.scalar]
            for kc in range(NKC):
                qs[kc % 2].dma_start(
                    out=wl_sb[:, kc, :], in_=wl[:, kc, :])
            for kc in range(NKC):
                qs[kc % 2].dma_start(
                    out=xT_sb[:, kc, 0:512], in_=xT[:, kc, 0:512])
            dma.dma_start(out=bcol_sb, in_=bcol[:, :])
            nc.scalar.dma_start(out=vb_sb, in_=vbias[:, :])
            dma.dma_start(out=ms_sb, in_=m_strict[:, :])
            nc.scalar.dma_start(out=mi_sb, in_=m_bias[:, :])
            dma.dma_start(out=ones_sb, in_=ones_in[:, 0:1])
            for n in range(4):
                nin = ones_in[:, 12 * n:12 * n + 12]
                nin = nin.rearrange("p (a b) -> p a b", a=HPG).unsqueeze(3)
                qs[n % 2].dma_start(out=vaug_n[n][:, :, :, 64:65], in_=nin)
            for n in range(1, 4):
                for kc in range(NKC):
                    qs[(n * NKC + kc) % 2].dma_start(
                        out=xT_sb[:, kc, n * 512:(n + 1) * 512],
                        in_=xT[:, kc, n * 512:(n + 1) * 512])
            dma.dma_start(out=wp01_sb, in_=wp01[:, :])
            nc.scalar.dma_start(out=wp2_sb, in_=wp2[:, :])

            def col(c0, n):
                m, p = divmod(c0, 128)
                return qkv_n[n][p:p + 64, m, :]

            # column layout pairs each matmul's lhsT/rhs at the same
            # partition base (hardware constraint)
            def qT(h, n):
                return col((0, 64, 256)[h], n)

            def kT(h, n):
                return col((128, 192, 384)[h], n)

            def q0T(n):
                return col(320, n)

            def k0T(n):
                return col(448, n)

            # ---- phase 1: q/k in [col, t] layout; v directly in natural
            # [t, d] layout (lhsT = xT chunk), all with bias folded in
            def emit_qkv_piece(n, piece):
                if piece < 4:
                    m = piece
                    qkv_ps = aps.tile([128, 512], F32, tag="att", name="qkv_ps")
                    for kc in range(NKC):
                        nc.tensor.matmul(
                            qkv_ps,
                            wl_sb[:, kc, m * 128:(m + 1) * 128],
                            xT_sb[:, kc, n * 512:(n + 1) * 512],
                            start=(kc == 0), stop=(kc == NKC - 1))
                    nc.scalar.activation(
                        qkv_n[n][:, m, :], qkv_ps,
                        mybir.ActivationFunctionType.Identity,
                        bias=bcol_sb[:, m:m + 1], scale=1.0)
                else:
                    ii = piece - 4
                    t0 = (4 * n + ii) * 128
                    v_ps = aps.tile([128, 192], F32, tag="att", name="v_ps")
                    for kc in range(NKC):
                        nc.tensor.matmul(
                            v_ps, xT_sb[:, kc, t0:t0 + 128],
                            wl_sb[:, kc, 512:704],
                            start=(kc == 0), stop=(kc == NKC - 1))
                    nc.vector.tensor_add(
                        vaug_n[n][:, :, ii, 0:64],
                        v_ps.rearrange("p (h d) -> p h d", h=HPG),
                        vb_sb.rearrange("p (h d) -> p h d", h=HPG))

            def emit_qkv(n):
                for piece in range(8):
                    emit_qkv_piece(n, piece)

            emit_qkv(0)

            # ---- phase 2: attention + FF, transposed layout ----
            # qkv production of chunk n+1 is emitted between attention
            # q-groups so the PE always has dense independent matmul work
            # (keeps the HAM clock gate open)
            for jg in range(NQG):
                if jg + 1 < 4:
                    emit_qkv(jg + 1)
                nt = 4 * jg + 4
                y_ps = [yps.tile([65, 512], F32, tag=f"y{h}", name=f"y_ps{h}")
                        for h in range(HPG)]
                ff_ps = fpp.tile([128, 512], F32, tag="fp", name="ff_ps")
                prev = None   # (i, qsl, ni, ti, pts, clip) pending AV/ff matmuls

                def emit_delayed(pi, pqsl, pni, pti, ppts, pclip):
                    nc.tensor.matmul(ff_ps[0:1, pqsl], ones_sb, pclip[:, pqsl],
                                     start=(pi == 0), stop=(pi == nt - 1))
                    for h in range(HPG):
                        nc.tensor.matmul(y_ps[h][:, pqsl],
                                         vaug_n[pni][:, h, pti, :],
                                         ppts[h][:, pqsl],
                                         start=(pi == 0), stop=(pi == nt - 1))

                for i in range(nt):
                    s0 = max(0, i - 4 * jg)
                    qoff = s0 * 128
                    qsl = slice(qoff, 512)        # local q cols within group
                    ni, ti = divmod(i, 4)
                    tsl = slice(ti * 128, (ti + 1) * 128)
                    # aux: head-0 scores -> S_T -> FF via scan
                    a0 = aps.tile([128, 512], F32, tag="att", name="a0")
                    nc.tensor.matmul(a0[:, qsl], k0T(ni)[:, tsl],
                                     q0T(jg)[:, qsl], start=True, stop=True)
                    st = p2.tile([128, 512], F32, tag="st")
                    nc.vector.tensor_scalar_max(st[:, qsl], a0[:, qsl], 0.0)
                    if i >= 4 * jg:
                        nc.vector.tensor_mul(st[:, qoff:qoff + 128],
                                             st[:, qoff:qoff + 128], ms_sb)
                    if i == 0:
                        nc.gpsimd.memset(st[0:1, :], 0.0)
                    incl = p2.tile([128, 513], F32, tag="incl")
                    nc.gpsimd.tensor_copy(incl[:, qoff:qoff + 1], accs[:, i:i + 1])
                    nc.vector.tensor_tensor_scan(
                        incl[:, qoff + 1:513], st[:, qsl], st[:, qsl],
                        initial=incl[:, qoff:qoff + 1],
                        op0=mybir.AluOpType.add, op1=mybir.AluOpType.bypass)
                    nc.gpsimd.tensor_copy(accs[:, i:i + 1], incl[:, 512:513])
                    enf = p2b.tile([128, 512], BF16, tag="enf")
                    nc.scalar.activation(enf[:, qsl], incl[:, qoff:512],
                                         mybir.ActivationFunctionType.Exp,
                                         scale=-1.0)
                    if i >= 4 * jg:
                        nc.vector.tensor_mul(enf[:, qoff:qoff + 128],
                                             enf[:, qoff:qoff + 128], mi_sb)
                    clip = p2b.tile([128, 512], BF16, tag="clip")
                    nc.vector.tensor_scalar_min(clip[:, qsl], incl[:, qoff:512], 1.0)
                    # heads: QK matmuls + exp + enf-mult; AV/ff of the
                    # previous chunk are emitted between them so the PE
                    # always has independent work during the ACT/DVE
                    # roundtrip (keeps the HAM clock gate open)
                    pts = []
                    for h in range(HPG):
                        ah = aps.tile([128, 512], F32, tag="att", name="ah")
                        nc.tensor.matmul(ah[:, qsl], kT(h, ni)[:, tsl],
                                         qT(h, jg)[:, qsl], start=True, stop=True)
                        pt = p2b.tile([128, 512], BF16, tag="pt")
                        nc.scalar.activation(pt[:, qsl], ah[:, qsl],
                                             mybir.ActivationFunctionType.Exp)
                        nc.vector.tensor_mul(pt[:, qsl], pt[:, qsl], enf[:, qsl])
                        pts.append(pt)
                    if prev is not None:
                        emit_delayed(*prev)
                    prev = (i, qsl, ni, ti, pts, clip)
                emit_delayed(*prev)
                prev = None
                # normalize + stash ffsum
                nc.vector.tensor_copy(ffs_sb[0:1, jg * 512:(jg + 1) * 512],
                                      ff_ps[0:1, :])
                for h in range(HPG):
                    zrow = zsb.tile([1, 512], F32, tag="zrow")
                    nc.scalar.copy(zrow, y_ps[h][64:65, :])
                    zi = zsb.tile([1, 512], F32, tag="zi")
                    nc.vector.reciprocal_approx_fast(zi, zrow)
                    zb = zsb.tile([64, 512], F32, tag="zb")
                    nc.gpsimd.partition_broadcast(zb, zi)
                    if h < 2:
                        dst = yT01[64 * h:64 * h + 64, jg * 512:(jg + 1) * 512]
                    else:
                        dst = yT2[:, jg * 512:(jg + 1) * 512]
                    nc.vector.tensor_mul(dst, y_ps[h][0:64, :], zb)
                # ---- proj for this q-group ----
                for s in range(4):
                    qt = jg * 4 + s
                    qsl2 = slice(qt * 128, (qt + 1) * 128)
                    out_sb = osb.tile([128, 768], BF16, tag="out")
                    for n0, nw in ((0, 512), (512, 256)):
                        pp = fpp.tile([128, 512], F32, tag="fp", name="pp")
                        nc.tensor.matmul(pp[:, 0:nw], yT01[:, qsl2],
                                         wp01_sb[:, n0:n0 + nw],
                                         start=True, stop=False)
                        nc.tensor.matmul(pp[:, 0:nw], yT2[:, qsl2],
                                         wp2_sb[:, n0:n0 + nw],
                                         start=False, stop=True)
                        nc.scalar.copy(out_sb[:, n0:n0 + nw], pp[:, 0:nw])
                    dma.dma_start(out=YP[qsl2, :], in_=out_sb)
            dma.dma_start(out=FFS[:, :], in_=ffs_sb)
    nc.finalize()
    return nc


_NC = None


def _get_nc():
    global _NC
    if _NC is None:
        _NC = _build_nc()
    return _NC


def _prep_inputs(x, w_attn, b_attn, w_proj):
    f32 = np.float32
    bf16 = np.dtype("bfloat16") if hasattr(np, "bfloat16") else None
    import ml_dtypes
    bf16 = ml_dtypes.bfloat16
    tril = np.tril(np.ones((128, 128), f32), k=-1)
    m_strict = np.ascontiguousarray(tril.T)          # 1 where col > row
    m_bias = np.ascontiguousarray(np.triu(np.ones((128, 128), f32))).astype(bf16)
    ones_in = np.ones((128, HPG * NTC), f32).astype(bf16)

    in_maps = []
    for c in range(8):
        b, g = divmod(c, 4)

        def qc(h):
            return slice(64 * (3 * g + h), 64 * (3 * g + h) + 64)

        def kc_(h):
            return slice(768 + 64 * (3 * g + h), 768 + 64 * (3 * g + h) + 64)

        def vc(h):
            return slice(1536 + 64 * (3 * g + h), 1536 + 64 * (3 * g + h) + 64)

        wl = np.zeros((768, 768), f32)
        bl = np.zeros((768,), f32)
        # column layout: [q0 q1 | k0 k1 | q2 aux_q0 | k2 aux_k0 | v0 v1 | v2 pad]
        pieces = [
            (0, qc(0), 0.125), (64, qc(1), 0.125),
            (128, kc_(0), 1.0), (192, kc_(1), 1.0),
            (256, qc(2), 0.125), (320, slice(0, 64), 0.125),
            (384, kc_(2), 1.0), (448, slice(768, 832), 1.0),
            (512, vc(0), 1.0), (576, vc(1), 1.0), (640, vc(2), 1.0),
        ]
        for dst, src, scale in pieces:
            wl[:, dst:dst + 64] = w_attn[:, src] * scale
            bl[dst:dst + 64] = b_attn[src] * scale
        in_maps.append({
            "xT": np.ascontiguousarray(
                x[b].T.reshape(NKC, 128, T).transpose(1, 0, 2)).astype(bf16),
            "wl": np.ascontiguousarray(
                wl.reshape(NKC, 128, 768).transpose(1, 0, 2)).astype(bf16),
            "bcol": np.ascontiguousarray(bl.reshape(NKC, 128).T),
            "wp01": w_proj[192 * g:192 * g + 128, :].astype(bf16),
            "wp2": w_proj[192 * g + 128:192 * g + 192, :].astype(bf16),
            "m_strict": m_strict,
            "m_bias": m_bias,
            "ones_in": ones_in,
            "vbias": np.ascontiguousarray(
                np.broadcast_to(bl[512:704], (128, 192))),
        })
    return in_maps


def run_device(x, w_attn, b_attn, w_proj, trace=False, trace_kwargs=None):
    nc = _get_nc()
    in_maps = _prep_inputs(x, w_attn, b_attn, w_proj)
    res = run_bass_kernel_spmd(nc, in_maps, core_ids=list(range(8)),
                               trace=trace, **(trace_kwargs or {}))
    return res


def kernel(x, w_attn, b_attn, w_proj, b_proj):
    x = np.asarray(x, np.float32)
    w_attn = np.asarray(w_attn, np.float32)
    b_attn = np.asarray(b_attn, np.float32)
    w_proj = np.asarray(w_proj, np.float32)
    b_proj = np.asarray(b_proj, np.float32)

    res = run_device(x, w_attn, b_attn, w_proj)
    y = np.zeros((B, T, 768), np.float32)
    ffsum = np.zeros((B, T), np.float32)
    for c in range(8):
        b = c // 4
        y[b] += res.results[c]["ypart"].astype(np.float32)
        if c % 4 == 0:
            ffsum[b] = res.results[c]["ffsum"][0]
    y += b_proj
    pos = np.arange(T, dtype=np.float32).reshape(1, 1, T, 1)
    M = pos - (ffsum / TAU).reshape(1, B, 1, T)
    return (y, M)


# revision 64
# speedup vs baseline: 1.1299x; 1.0050x over previous
"""Trainium2 Bass kernel for CausalSelectiveSelfAttentionWithMemoryPenalty.

Reference computation (B=2, T=2048, C=768, nh=12, hd=64, tau=1.0):
    qkv = x @ w_attn + b_attn ; q,k,v per head
    att = causal_masked(q k^T / 8)
    S = relu(att[:,0]) with col 0 and diagonal zeroed
    FF[q,t] = sum_{r<q} S[r,t]          (cumsum of row-shifted S)
    FF_sum[q] = sum_t clip(FF[q,t],0,1) ;  M[0,b,i,j] = i - FF_sum[b,j]
    y = softmax(att - FF) @ v ; y = y @ w_proj + b_proj
    returns (y, M)

Sharding: 8 cores = 2 batches x 4 head-groups (3 heads each). Each core
recomputes head-0 scores for its batch (aux slot) to build FF locally —
no collectives. Attention is computed transposed (attT[t,q]) so that
softmax normalization comes from an appended ones-column of v and the
AV matmul consumes P_T directly as the moving operand. M is built on
host from FF_sum (tiny) instead of shipping 33MB from the device.

Numerics: matmul/elementwise P path in bf16 (PE 1 cyc/row; DVE
tensor_tensor 2x mode is bf16-only), FF/scan chain in fp32. qkv and v
tensors are split into per-512-column tiles so the Tile scheduler can
overlap qkv production with early attention chunks (keeps the PE dense
enough that the HAM clock gate stays open).
"""
import sys

if "/opt/trn_rl_repo" not in sys.path:
    sys.path.insert(0, "/opt/trn_rl_repo")

import numpy as np

import concourse.bass as bass
from concourse import bacc, mybir
from concourse.tile import TileContext
from concourse.bass_utils import run_bass_kernel_spmd

F32 = mybir.dt.float32
BF16 = mybir.dt.bfloat16

B, T, C = 2, 2048, 768
NH, HD = 12, 64
TAU = 1.0
HPG = 3           # heads per group
NKC = C // 128    # 6 contraction chunks for qkv
NQG = T // 512    # 4 q-groups of 512
NTC = T // 128    # 16 t-chunks of 128


def _build_nc():
    nc = bacc.Bacc()

    xT = nc.dram_tensor("xT", [128, NKC, T], BF16, kind="ExternalInput")
    wl = nc.dram_tensor("wl", [128, NKC, 768], BF16, kind="ExternalInput")
    bcol = nc.dram_tensor("bcol", [128, NKC], F32, kind="ExternalInput")
    wp01 = nc.dram_tensor("wp01", [128, 768], BF16, kind="ExternalInput")
    wp2 = nc.dram_tensor("wp2", [64, 768], BF16, kind="ExternalInput")
    m_strict = nc.dram_tensor("m_strict", [128, 128], BF16, kind="ExternalInput")
    m_bias = nc.dram_tensor("m_bias", [128, 128], BF16, kind="ExternalInput")
    ones_in = nc.dram_tensor("ones_in", [128, HPG * NTC], BF16, kind="ExternalInput")
    vbias = nc.dram_tensor("vbias", [128, 192], F32, kind="ExternalInput")

    YP = nc.dram_tensor("ypart", [T, 768], BF16, kind="ExternalOutput")
    FFS = nc.dram_tensor("ffsum", [1, T], F32, kind="ExternalOutput")

    with TileContext(nc) as tc:
        dma = nc.sync
        with tc.tile_pool(name="const", bufs=1) as constp, \
             tc.tile_pool(name="pers", bufs=1) as pers, \
             tc.tile_pool(name="ph2sb", bufs=4) as p2, \
             tc.tile_pool(name="ptsb", bufs=8) as p2b, \
             tc.tile_pool(name="zsb", bufs=2) as zsb, \
             tc.tile_pool(name="osb", bufs=2) as osb, \
             tc.tile_pool(name="aps", bufs=4, space="PSUM") as aps, \
             tc.tile_pool(name="yps", bufs=1, space="PSUM") as yps, \
             tc.tile_pool(name="fpp", bufs=1, space="PSUM") as fpp:
            ms_sb = constp.tile([128, 128], BF16)
            mi_sb = constp.tile([128, 128], BF16)
            ones_sb = constp.tile([128, 1], BF16)
            wp01_sb = constp.tile([128, 768], BF16)
            wp2_sb = constp.tile([64, 768], BF16)
            bcol_sb = constp.tile([128, NKC], F32)
            vb_sb = constp.tile([128, 192], F32)

            # persistent state; qkv/vaug split per 512-wide t-chunk so early
            # attention chunks only depend on the slices already produced
            qkv_n = [pers.tile([128, 4, 512], BF16, name=f"qkv{n}")
                     for n in range(4)]
            vaug_n = [pers.tile([128, HPG, 4, 65], BF16, name=f"vaug{n}")
                      for n in range(4)]
            xT_sb = pers.tile([128, NKC, T], BF16)
            wl_sb = pers.tile([128, NKC, 768], BF16)
            yT01 = pers.tile([128, T], BF16)          # heads 0,1 (d stacked)
            yT2 = pers.tile([64, T], BF16)            # head 2
            accs = pers.tile([128, NTC], BF16)         # cumsum carry per t-chunk
            ffs_sb = pers.tile([1, T], F32)
            nc.vector.memset(accs, 0.0)
            # input DMAs spread over both HWDGE queues in consumption order:
            # weights + first x chunk first, early-needed constants, then the
            # remaining x chunks and late-use constants
            qs = [nc.sync, nc.scalar]
            for kc in range(NKC):
                qs[kc % 2].dma_start(
                    out=wl_sb[:, kc, :], in_=wl[:, kc, :])
            for kc in range(NKC):
                qs[kc % 2].dma_start(
                    out=xT_sb[:, kc, 0:512], in_=xT[:, kc, 0:512])
            dma.dma_start(out=bcol_sb, in_=bcol[:, :])
            nc.scalar.dma_start(out=vb_sb, in_=vbias[:, :])
            dma.dma_start(out=ms_sb, in_=m_strict[:, :])
            nc.scalar.dma_start(out=mi_sb, in_=m_bias[:, :])
            dma.dma_start(out=ones_sb, in_=ones_in[:, 0:1])
            for n in range(4):
                nin = ones_in[:, 12 * n:12 * n + 12]
                nin = nin.rearrange("p (a b) -> p a b", a=HPG).unsqueeze(3)
                qs[n % 2].dma_start(out=vaug_n[n][:, :, :, 64:65], in_=nin)
            for n in range(1, 4):
                for kc in range(NKC):
                    qs[(n * NKC + kc) % 2].dma_start(
                        out=xT_sb[:, kc, n * 512:(n + 1) * 512],
                        in_=xT[:, kc, n * 512:(n + 1) * 512])
            dma.dma_start(out=wp01_sb, in_=wp01[:, :])
            nc.scalar.dma_start(out=wp2_sb, in_=wp2[:, :])

            def col(c0, n):
                m, p = divmod(c0, 128)
                return qkv_n[n][p:p + 64, m, :]

            # column layout pairs each matmul's lhsT/rhs at the same
            # partition base (hardware constraint)
            def qT(h, n):
                return col((0, 64, 256)[h], n)

            def kT(h, n):
                return col((128, 192, 384)[h], n)

            def q0T(n):
                return col(320, n)

            def k0T(n):
                return col(448, n)

            # ---- phase 1: q/k in [col, t] layout; v directly in natural
            # [t, d] layout (lhsT = xT chunk), all with bias folded in
            def emit_qkv_piece(n, piece):
                if piece < 4:
                    m = piece
                    qkv_ps = aps.tile([128, 512], F32, tag="att", name="qkv_ps")
                    for kc in range(NKC):
                        nc.tensor.matmul(
                            qkv_ps,
                            wl_sb[:, kc, m * 128:(m + 1) * 128],
                            xT_sb[:, kc, n * 512:(n + 1) * 512],
                            start=(kc == 0), stop=(kc == NKC - 1))
                    nc.scalar.activation(
                        qkv_n[n][:, m, :], qkv_ps,
                        mybir.ActivationFunctionType.Identity,
                        bias=bcol_sb[:, m:m + 1], scale=1.0)
                else:
                    ii = piece - 4
                    t0 = (4 * n + ii) * 128
                    v_ps = aps.tile([128, 192], F32, tag="att", name="v_ps")
                    for kc in range(NKC):
                        nc.tensor.matmul(
                            v_ps, xT_sb[:, kc, t0:t0 + 128],
                            wl_sb[:, kc, 512:704],
                            start=(kc == 0), stop=(kc == NKC - 1))
                    nc.vector.tensor_add(
                        vaug_n[n][:, :, ii, 0:64],
                        v_ps.rearrange("p (h d) -> p h d", h=HPG),
                        vb_sb.rearrange("p (h d) -> p h d", h=HPG))

            def emit_qkv(n):
                for piece in range(8):
                    emit_qkv_piece(n, piece)

            emit_qkv(0)

            # ---- phase 2: attention + FF, transposed layout ----
            # qkv production of chunk n+1 is emitted between attention
            # q-groups so the PE always has dense independent matmul work
            # (keeps the HAM clock gate open)
            for jg in range(NQG):
                if jg + 1 < 4:
                    emit_qkv(jg + 1)
                nt = 4 * jg + 4
                y_ps = [yps.tile([65, 512], F32, tag=f"y{h}", name=f"y_ps{h}")
                        for h in range(HPG)]
                ff_ps = fpp.tile([128, 512], F32, tag="fp", name="ff_ps")
                prev = None   # (i, qsl, ni, ti, pts, clip) pending AV/ff matmuls

                def emit_delayed(pi, pqsl, pni, pti, ppts, pclip):
                    nc.tensor.matmul(ff_ps[0:1, pqsl], ones_sb, pclip[:, pqsl],
                                     start=(pi == 0), stop=(pi == nt - 1))
                    for h in range(HPG):
                        nc.tensor.matmul(y_ps[h][:, pqsl],
                                         vaug_n[pni][:, h, pti, :],
                                         ppts[h][:, pqsl],
                                         start=(pi == 0), stop=(pi == nt - 1))

                for i in range(nt):
                    s0 = max(0, i - 4 * jg)
                    qoff = s0 * 128
                    qsl = slice(qoff, 512)        # local q cols within group
                    ni, ti = divmod(i, 4)
                    tsl = slice(ti * 128, (ti + 1) * 128)
                    # aux: head-0 scores -> S_T -> FF via scan
                    a0 = aps.tile([128, 512], F32, tag="att", name="a0")
                    nc.tensor.matmul(a0[:, qsl], k0T(ni)[:, tsl],
                                     q0T(jg)[:, qsl], start=True, stop=True)
                    st = p2.tile([128, 512], BF16, tag="st")
                    nc.vector.tensor_scalar_max(st[:, qsl], a0[:, qsl], 0.0)
                    if i >= 4 * jg:
                        nc.vector.tensor_mul(st[:, qoff:qoff + 128],
                                             st[:, qoff:qoff + 128], ms_sb)
                    if i == 0:
                        nc.gpsimd.memset(st[0:1, :], 0.0)
                    incl = p2.tile([128, 513], BF16, tag="incl")
                    nc.gpsimd.tensor_copy(incl[:, qoff:qoff + 1], accs[:, i:i + 1])
                    nc.vector.tensor_tensor_scan(
                        incl[:, qoff + 1:513], st[:, qsl], st[:, qsl],
                        initial=incl[:, qoff:qoff + 1],
                        op0=mybir.AluOpType.add, op1=mybir.AluOpType.bypass)
                    nc.gpsimd.tensor_copy(accs[:, i:i + 1], incl[:, 512:513])
                    enf = p2b.tile([128, 512], BF16, tag="enf")
                    nc.scalar.activation(enf[:, qsl], incl[:, qoff:512],
                                         mybir.ActivationFunctionType.Exp,
                                         scale=-1.0)
                    if i >= 4 * jg:
                        nc.vector.tensor_mul(enf[:, qoff:qoff + 128],
                                             enf[:, qoff:qoff + 128], mi_sb)
                    clip = p2b.tile([128, 512], BF16, tag="clip")
                    nc.vector.tensor_scalar_min(clip[:, qsl], incl[:, qoff:512], 1.0)
                    # heads: QK matmuls + exp + enf-mult; AV/ff of the
                    # previous chunk are emitted between them so the PE
                    # always has independent work during the ACT/DVE
                    # roundtrip (keeps the HAM clock gate open)
                    pts = []
                    for h in range(HPG):
                        ah = aps.tile([128, 512], F32, tag="att", name="ah")
                        nc.tensor.matmul(ah[:, qsl], kT(h, ni)[:, tsl],
                                         qT(h, jg)[:, qsl], start=True, stop=True)
                        pt = p2b.tile([128, 512], BF16, tag="pt")
                        nc.scalar.activation(pt[:, qsl], ah[:, qsl],
                                             mybir.ActivationFunctionType.Exp)
                        nc.vector.tensor_mul(pt[:, qsl], pt[:, qsl], enf[:, qsl])
                        pts.append(pt)
                    if prev is not None:
                        emit_delayed(*prev)
                    prev = (i, qsl, ni, ti, pts, clip)
                emit_delayed(*prev)
                prev = None
                # normalize + stash ffsum
                nc.vector.tensor_copy(ffs_sb[0:1, jg * 512:(jg + 1) * 512],
                                      ff_ps[0:1, :])
                for h in range(HPG):
                    zrow = zsb.tile([1, 512], F32, tag="zrow")
                    nc.scalar.copy(zrow, y_ps[h][64:65, :])
                    zi = zsb.tile([1, 512], F32, tag="zi")
                    nc.vector.reciprocal_approx_fast(zi, zrow)
                    zb = zsb.tile([64, 512], F32, tag="zb")
                    nc.gpsimd.partition_broadcast(zb, zi)
                    if h < 2:
                        dst = yT01[64 * h:64 * h + 64, jg * 512:(jg + 1) * 512]
                    else:
                        dst = yT2[:, jg * 512:(jg + 1) * 512]
                    nc.vector.tensor_mul(dst, y_ps[h][0:64, :], zb)
                # ---- proj for this q-group ----
                for s in range(4):
                    qt = jg * 4 + s
                    qsl2 = slice(qt * 128, (qt + 1) * 128)
                    out_sb = osb.tile([128, 768], BF16, tag="out")
                    for n0, nw in ((0, 512), (512, 256)):
                        pp = fpp.tile([128, 512], F32, tag="fp", name="pp")
                        nc.tensor.matmul(pp[:, 0:nw], yT01[:, qsl2],
                                         wp01_sb[:, n0:n0 + nw],
                                         start=True, stop=False)
                        nc.tensor.matmul(pp[:, 0:nw], yT2[:, qsl2],
                                         wp2_sb[:, n0:n0 + nw],
                                         start=False, stop=True)
                        nc.scalar.copy(out_sb[:, n0:n0 + nw], pp[:, 0:nw])
                    dma.dma_start(out=YP[qsl2, :], in_=out_sb)
            dma.dma_start(out=FFS[:, :], in_=ffs_sb)
    nc.finalize()
    return nc


_NC = None


def _get_nc():
    global _NC
    if _NC is None:
        _NC = _build_nc()
    return _NC


def _prep_inputs(x, w_attn, b_attn, w_proj):
    f32 = np.float32
    bf16 = np.dtype("bfloat16") if hasattr(np, "bfloat16") else None
    import ml_dtypes
    bf16 = ml_dtypes.bfloat16
    tril = np.tril(np.ones((128, 128), f32), k=-1)
    m_strict = np.ascontiguousarray(tril.T).astype(bf16)          # 1 where col > row
    m_bias = np.ascontiguousarray(np.triu(np.ones((128, 128), f32))).astype(bf16)
    ones_in = np.ones((128, HPG * NTC), f32).astype(bf16)

    in_maps = []
    for c in range(8):
        b, g = divmod(c, 4)

        def qc(h):
            return slice(64 * (3 * g + h), 64 * (3 * g + h) + 64)

        def kc_(h):
            return slice(768 + 64 * (3 * g + h), 768 + 64 * (3 * g + h) + 64)

        def vc(h):
            return slice(1536 + 64 * (3 * g + h), 1536 + 64 * (3 * g + h) + 64)

        wl = np.zeros((768, 768), f32)
        bl = np.zeros((768,), f32)
        # column layout: [q0 q1 | k0 k1 | q2 aux_q0 | k2 aux_k0 | v0 v1 | v2 pad]
        pieces = [
            (0, qc(0), 0.125), (64, qc(1), 0.125),
            (128, kc_(0), 1.0), (192, kc_(1), 1.0),
            (256, qc(2), 0.125), (320, slice(0, 64), 0.125),
            (384, kc_(2), 1.0), (448, slice(768, 832), 1.0),
            (512, vc(0), 1.0), (576, vc(1), 1.0), (640, vc(2), 1.0),
        ]
        for dst, src, scale in pieces:
            wl[:, dst:dst + 64] = w_attn[:, src] * scale
            bl[dst:dst + 64] = b_attn[src] * scale
        in_maps.append({
            "xT": np.ascontiguousarray(
                x[b].T.reshape(NKC, 128, T).transpose(1, 0, 2)).astype(bf16),
            "wl": np.ascontiguousarray(
                wl.reshape(NKC, 128, 768).transpose(1, 0, 2)).astype(bf16),
            "bcol": np.ascontiguousarray(bl.reshape(NKC, 128).T),
            "wp01": w_proj[192 * g:192 * g + 128, :].astype(bf16),
            "wp2": w_proj[192 * g + 128:192 * g + 192, :].astype(bf16),
            "m_strict": m_strict,
            "m_bias": m_bias,
            "ones_in": ones_in,
            "vbias": np.ascontiguousarray(
                np.broadcast_to(bl[512:704], (128, 192))),
        })
    return in_maps


def run_device(x, w_attn, b_attn, w_proj, trace=False, trace_kwargs=None):
    nc = _get_nc()
    in_maps = _prep_inputs(x, w_attn, b_attn, w_proj)
    res = run_bass_kernel_spmd(nc, in_maps, core_ids=list(range(8)),
                               trace=trace, **(trace_kwargs or {}))
    return res


def kernel(x, w_attn, b_attn, w_proj, b_proj):
    x = np.asarray(x, np.float32)
    w_attn = np.asarray(w_attn, np.float32)
    b_attn = np.asarray(b_attn, np.float32)
    w_proj = np.asarray(w_proj, np.float32)
    b_proj = np.asarray(b_proj, np.float32)

    res = run_device(x, w_attn, b_attn, w_proj)
    y = np.zeros((B, T, 768), np.float32)
    ffsum = np.zeros((B, T), np.float32)
    for c in range(8):
        b = c // 4
        y[b] += res.results[c]["ypart"].astype(np.float32)
        if c % 4 == 0:
            ffsum[b] = res.results[c]["ffsum"][0]
    y += b_proj
    pos = np.arange(T, dtype=np.float32).reshape(1, 1, T, 1)
    M = pos - (ffsum / TAU).reshape(1, B, 1, T)
    return (y, M)
